# revision 1
# baseline (speedup 1.0000x reference)
"""GNN message passing (nn_OPID_78769700208710) on 8 TRN2 NeuronCores.

Strategy: the 6-relation edge lists are combined on host into one sparse
operator M (w[e] = sign_r * softplus(g_r) * val[e]), materialized as a dense
fp16 matrix A [N_pad, N_pad] (N_pad = 20480).  Propagation h_{k+1} =
a_k*h0 + (1-a_k)*(h @ A) runs 6 steps on device.

Sharding: destination-column model parallelism.  Core c owns dst columns
[c*2560, (c+1)*2560) and streams its A slice (panels of [128 src, 2560 dst]
fp16) from HBM each step; matmuls accumulate msg in PSUM.  Steps 1-5 produce
msg in node-partition layout ([128 dst, 64 batch] per dst-block) so the next
step's stationary operand (h windows, [128 src, 64 b]) needs no transpose;
h slices are exchanged between steps with an in-kernel AllGather.  Step 6
produces msg in batch-partition layout [64, 2560] feeding the decode, which
runs fully on-device: a K=4 matmul folds W1+b1 (4th input row is ones), ACT
relu, then a column-stationary matmul against W2 sums over H.  cell_emb is
added AFTER the relu in the reference, so it passes linearly through W2 and
is folded host-side into a per-output-column bias together with b2.
"""

import numpy as np

N = 20000
NP = 20480          # padded nodes: 160 windows * 128
W = 160             # src windows of 128
B = 64              # batch
CORES = 8
NLOC = NP // CORES  # 2560 dst nodes per core
WLOC = NLOC // 128  # 20 dst blocks per core
H = 64
STEPS = 6
SIGNS = (1.0, -1.0, 1.0, -1.0, 1.0, -1.0)

_CACHE = {}


def _np_softplus(x):
    return np.log1p(np.exp(-np.abs(x))) + np.maximum(x, 0.0)


def _np_sigmoid(x):
    return 1.0 / (1.0 + np.exp(-x))


def _build_program(NP=NP, debug=False, compile_=True):
    """Build + compile the (input-independent) Bass program once."""
    key = ("nc", NP, debug)
    if key in _CACHE:
        return _CACHE[key]
    W = NP // 128
    NLOC = NP // CORES
    WLOC = NLOC // 128

    import concourse.bacc as bacc
    import concourse.mybir as mybir
    from concourse import tile

    f16 = mybir.dt.float16
    f32 = mybir.dt.float32
    AF = mybir.ActivationFunctionType
    OP = mybir.AluOpType

    nc = bacc.Bacc(
        "TRN2",
        target_bir_lowering=False,
        debug=False,
        enable_asserts=False,
        num_devices=CORES,
    )

    a2 = nc.dram_tensor("a2", [W, 128, NLOC], f16, kind="ExternalInput")
    h0t16 = nc.dram_tensor("h0t16", [128, W * B], f16, kind="ExternalInput")
    h0t = nc.dram_tensor("h0t", [128, WLOC * B], f32, kind="ExternalInput")
    x4b = nc.dram_tensor("x4b", [B, 4 * NLOC], f16, kind="ExternalInput")
    w1bT = nc.dram_tensor("w1bT", [4, B * H], f16, kind="ExternalInput")
    w2sc = nc.dram_tensor("w2sc", [H, 1], f32, kind="ExternalInput")
    b2bc = nc.dram_tensor("b2bc", [128, WLOC * B], f32, kind="ExternalInput")
    alph = nc.dram_tensor("alph", [128, 2 * STEPS], f32, kind="ExternalInput")
    y = nc.dram_tensor("y", [B, NLOC], f32, kind="ExternalOutput")
    if debug:
        dbg_h = nc.dram_tensor("dbg_h", [STEPS - 1, 128, W * B], f16, kind="ExternalOutput")
        dbg_x = nc.dram_tensor("dbg_x", [B, 4 * NLOC], f16, kind="ExternalOutput")

    NCHUNK = NLOC // 512  # 5

    with tile.TileContext(nc) as tc:
        with (
            tc.tile_pool(name="const", bufs=1) as constp,
            tc.tile_pool(name="apan", bufs=4) as apanp,
            tc.tile_pool(name="hslice", bufs=2) as hslicep,
            tc.tile_pool(name="tmp", bufs=4) as tmpp,
            tc.tile_pool(name="dec", bufs=2) as decp,
            tc.tile_pool(name="hds", bufs=4) as hdsp,
            tc.tile_pool(name="ysb", bufs=2) as ysbp,
            tc.tile_pool(name="dram", bufs=1, space="DRAM") as dramp,
        ):
            # --- persistent SBUF state ---
            h_sb = constp.tile([128, W * B], f16, tag="h_sb")
            h0t_sb = constp.tile([128, WLOC * B], f32, tag="h0t")
            alph_sb = constp.tile([128, 2 * STEPS], f32, tag="alph")
            w2_sb = constp.tile([H, 1], f32, tag="w2")
            b2_sb = constp.tile([128, WLOC * B], f32, tag="b2")
            w1b_sb = constp.tile([4, B * H], f16, tag="w1b")
            w2c_sb = constp.tile([H, 1], f16, tag="w2c")
            xsb = constp.tile([B, 4 * NLOC], f16, tag="xsb")

            nc.sync.dma_start(h_sb[:], h0t16.ap())
            nc.sync.dma_start(h0t_sb[:], h0t.ap())
            nc.sync.dma_start(alph_sb[:], alph.ap())
            nc.sync.dma_start(w2_sb[:], w2sc.ap())
            nc.sync.dma_start(b2_sb[:], b2bc.ap())
            nc.sync.dma_start(w1b_sb[:], w1bT.ap())
            nc.sync.dma_start(xsb[:], x4b.ap())
            nc.vector.tensor_copy(w2c_sb[:], w2_sb[:])

            # DRAM bounce buffers for the per-step h exchange
            bi = dramp.tile([128, WLOC * B], f16, tag="bi")
            bo = dramp.tile([CORES, 128, WLOC * B], f16, tag="bo")
            xd = dramp.tile([B, 4 * NLOC], f16, tag="xd")

            # ---------------- propagation steps 1..5 ----------------
            prop = tc.tile_pool(name="psprop", bufs=1, space="PSUM")
            ps15p = ps6p = prop.__enter__()
            for k in range(STEPS - 1):
                ps = [ps15p.tile([128, 512], f32, tag=f"ps15_{i}", name=f"ps15_{i}") for i in range(3)]
                h16s = hslicep.tile([128, WLOC * B], f16, tag="h16s")

                for w in range(W):
                    ap = apanp.tile([128, NLOC], f16, tag="apan")
                    nc.sync.dma_start(ap[:], a2.ap()[w])
                    for d in range(WLOC):
                        # one accumulation group per 2KB PSUM bank: start only
                        # on the bank's first matmul, stop on its last; other
                        # column-ranges are initialized via pending-zero bytes
                        nc.tensor.matmul(
                            ps[d // 8][:, (d % 8) * B : (d % 8 + 1) * B],
                            lhsT=ap[:, d * 128 : (d + 1) * 128],
                            rhs=h_sb[:, w * B : (w + 1) * B],
                            start=(w == 0 and d % 8 == 0),
                            stop=(w == W - 1 and (d % 8 == 7 or d == WLOC - 1)),
                        )

                # epilogue: h_new = a*h0 + (1-a)*msg, emitted as fp16
                for d in range(WLOC):
                    h0a = tmpp.tile([128, B], f32, tag="h0a")
                    nc.scalar.activation(
                        h0a[:],
                        h0t_sb[:, d * B : (d + 1) * B],
                        AF.Copy,
                        scale=alph_sb[:, k : k + 1],
                    )
                    nc.vector.scalar_tensor_tensor(
                        h16s[:, d * B : (d + 1) * B],
                        ps[d // 8][:, (d % 8) * B : (d % 8 + 1) * B],
                        alph_sb[:, STEPS + k : STEPS + k + 1],
                        h0a[:],
                        OP.mult,
                        OP.add,
                    )

                # exchange: slice -> DRAM -> AllGather -> full h_sb
                nc.sync.dma_start(bi[:], h16s[:])
                nc.gpsimd.collective_compute(
                    "AllGather",
                    OP.bypass,
                    replica_groups=[list(range(CORES))],
                    ins=[bi.opt()],
                    outs=[bo.opt()],
                )
                nc.sync.dma_start(
                    h_sb[:].rearrange("p (c f) -> p c f", c=CORES),
                    bo[:].rearrange("c p f -> p c f"),
                )
                if debug:
                    nc.sync.dma_start(dbg_h.ap()[k], h_sb[:])

            # ---------------- step 6: batch-partition output ----------------
            ps6 = [ps6p.tile([B, 512], f32, tag=f"ps6_{j}", name=f"ps6_{j}") for j in range(NCHUNK)]
            for w in range(W):
                ap = apanp.tile([128, NLOC], f16, tag="apan")
                nc.sync.dma_start(ap[:], a2.ap()[w])
                for j in range(NCHUNK):
                    nc.tensor.matmul(
                        ps6[j][:, :],
                        lhsT=h_sb[:, w * B : (w + 1) * B],
                        rhs=ap[:, j * 512 : (j + 1) * 512],
                        start=(w == 0),
                        stop=(w == W - 1),
                    )

            # epilogue 6 in batch layout, written into xsb row 2 (h6, fp16)
            k5 = STEPS - 1
            for j in range(NCHUNK):
                h0a6 = tmpp.tile([B, 512], f32, tag="h0a6")
                nc.scalar.activation(
                    h0a6[:],
                    xsb[:, NLOC + j * 512 : NLOC + (j + 1) * 512],
                    AF.Copy,
                    scale=alph_sb[:B, k5 : k5 + 1],
                )
                nc.vector.scalar_tensor_tensor(
                    xsb[:, 2 * NLOC + j * 512 : 2 * NLOC + (j + 1) * 512],
                    ps6[j][:, :],
                    alph_sb[:B, STEPS + k5 : STEPS + k5 + 1],
                    h0a6[:],
                    OP.mult,
                    OP.add,
                )

            prop.__exit__(None, None, None)

            # ---------------- decode ----------------
            decps = tc.tile_pool(name="psdec", bufs=1, space="PSUM")
            psAp = ps2p = decps.__enter__()
            nc.sync.dma_start(xd[:], xsb[:])
            if debug:
                nc.sync.dma_start(dbg_x.ap(), xsb[:])

            NQ = 8          # batch rounds
            BQ = B // NQ    # 8 batch rows per round
            ps2_tiles = [ps2p.tile([128, 512], f32, tag=f"ps2_{i}", name=f"ps2_{i}") for i in range(3)]
            ncols_done = 0
            ysb_flushed = 0
            NCOLS_TOT = B * NLOC // 128  # 1280

            for q in range(NQ):
                xT4 = decp.tile([4, BQ * NLOC], f16, tag="xT4")
                # gather [f, b, n] for this batch block from DRAM
                nc.sync.dma_start(
                    xT4[:].rearrange("f (b n) -> f b n", b=BQ),
                    xd[:].rearrange("b (f n) -> f b n", f=4)[:, q * BQ : (q + 1) * BQ, :],
                )
                for bl in range(BQ):
                    b = q * BQ + bl
                    for c5 in range(NCHUNK):
                        psA = psAp.tile([H, 512], f32, tag="psA", bufs=4)
                        nc.tensor.matmul(
                            psA[:],
                            lhsT=w1b_sb[:, b * H : (b + 1) * H],
                            rhs=xT4[
                                :, bl * NLOC + c5 * 512 : bl * NLOC + (c5 + 1) * 512
                            ],
                            start=True,
                            stop=True,
                        )
                        hds = hdsp.tile([H, 512], f16, tag="hds")
                        nc.scalar.activation(hds[:], psA[:], AF.Relu)
                        for i in range(4):
                            col = ncols_done % 512
                            ti = ncols_done // 512
                            nc.tensor.matmul(
                                ps2_tiles[ti][:, col : col + 1],
                                lhsT=hds[:, i * 128 : (i + 1) * 128],
                                rhs=w2c_sb[:],
                                start=True,
                                stop=True,
                            )
                            ncols_done += 1
                            if ncols_done % 512 == 0 or ncols_done == NCOLS_TOT:
                                nt = ncols_done - ysb_flushed
                                ysb = ysbp.tile([128, 512], f32, tag="ysb")
                                nc.vector.scalar_tensor_tensor(
                                    ysb[:, :nt],
                                    ps2_tiles[ti][:, :nt],
                                    1.0,
                                    b2_sb[:, ysb_flushed:ncols_done],
                                    OP.mult,
                                    OP.add,
                                )
                                dst = (
                                    y.ap()
                                    .rearrange("b n -> (b n)")[
                                        ysb_flushed * 128 : ncols_done * 128
                                    ]
                                    .rearrange("(f p) -> p f", p=128)
                                )
                                nc.sync.dma_start(dst, ysb[:, :nt])
                                ysb_flushed = ncols_done
            decps.__exit__(None, None, None)

    if compile_:
        nc.compile()
    _CACHE[key] = nc
    return nc


def kernel(
    ctl_base,
    u_raw,
    g_logits,
    alpha_logits,
    cell_emb,
    W1,
    b1,
    W2,
    b2,
    edge_val,
    edge_src,
    edge_dst,
    cell_idx,
):
    from concourse.bass_utils import run_bass_kernel_spmd

    ctl_base = np.asarray(ctl_base)
    u_raw = np.asarray(u_raw)
    cell_emb = np.asarray(cell_emb)
    W1 = np.asarray(W1)
    b1 = np.asarray(b1)
    W2 = np.asarray(W2)
    b2 = np.asarray(b2)
    edge_val = np.asarray(edge_val)
    edge_src = np.asarray(edge_src)
    edge_dst = np.asarray(edge_dst)
    cell_idx = np.asarray(cell_idx)

    g = _np_softplus(np.asarray(g_logits, np.float64))
    alphas = _np_sigmoid(np.asarray(alpha_logits, np.float64))

    # dense combined operator A[src, dst]
    A = np.zeros((NP, NP), np.float32)
    for r in range(6):
        w = (SIGNS[r] * g[r]) * np.asarray(edge_val[r], np.float64)
        np.add.at(A, (edge_src[r], edge_dst[r]), w.astype(np.float32))

    u_pad = np.zeros((B, NP), np.float32)
    u_pad[:, :N] = u_raw
    ctl_pad = np.zeros((B, NP), np.float32)
    ctl_pad[:, :N] = ctl_base

    # full transposed h0 in window layout: [p, w*B + b] = u[b, w*128+p]
    h0t16_full = np.ascontiguousarray(
        u_pad.reshape(B, W, 128).transpose(2, 1, 0).reshape(128, W * B)
    ).astype(np.float16)

    alph_np = np.zeros((128, 2 * STEPS), np.float32)
    alph_np[:, :STEPS] = alphas.astype(np.float32)
    alph_np[:, STEPS:] = (1.0 - alphas).astype(np.float32)

    cemb_rows = cell_emb[cell_idx]  # [B, H]
    w1bT_np = np.zeros((4, B * H), np.float16)
    for f in range(3):
        w1bT_np[f] = np.tile(W1[f].astype(np.float16), B)
    w1bT_np[3] = np.tile(b1.astype(np.float16), B)

    w2sc_np = np.ascontiguousarray(W2.reshape(H, 1)).astype(np.float32)
    # reference adds cell_emb AFTER the relu; it passes linearly through W2:
    # y += cemb[b] @ W2.  Fold per-batch constant + b2 into a per-column bias
    # (ps2 column col -> batch b = col // WLOC).
    ccb = (cemb_rows.astype(np.float64) @ W2.astype(np.float64).reshape(H)).astype(np.float32)  # [B]
    ncols_tot = B * WLOC
    bias_cols = (np.repeat(ccb, WLOC) + np.float32(b2.reshape(-1)[0])).astype(np.float32)  # [1280]
    b2bc_np = np.broadcast_to(bias_cols[None, :], (128, ncols_tot)).copy()

    nc = _build_program()

    W_, NLOC_, WLOC_ = W, NLOC, WLOC
    in_maps = []
    for c in range(CORES):
        sl = slice(c * NLOC, (c + 1) * NLOC)
        a2_c = np.ascontiguousarray(A.reshape(W, 128, NP)[:, :, sl]).astype(np.float16)
        h0t_c = np.ascontiguousarray(
            u_pad[:, sl].reshape(B, WLOC, 128).transpose(2, 1, 0).reshape(128, WLOC * B)
        ).astype(np.float32)
        x4b_c = np.zeros((B, 4, NLOC), np.float16)
        x4b_c[:, 0, :] = ctl_pad[:, sl].astype(np.float16)
        x4b_c[:, 1, :] = u_pad[:, sl].astype(np.float16)
        x4b_c[:, 3, :] = np.float16(1.0)
        in_maps.append(
            {
                "a2": a2_c,
                "h0t16": h0t16_full,
                "h0t": h0t_c,
                "x4b": x4b_c.reshape(B, 4 * NLOC),
                "w1bT": w1bT_np,
                "w2sc": w2sc_np,
                "b2bc": b2bc_np,
                "alph": alph_np,
            }
        )

    _CACHE["in_maps"] = in_maps
    res = run_bass_kernel_spmd(nc, in_maps, core_ids=list(range(CORES)))
    out = np.concatenate([res.results[c]["y"] for c in range(CORES)], axis=1)
    return np.ascontiguousarray(out[:, :N]).astype(np.float32)

